# Initial kernel scaffold
#
"""Trainium2 Bass kernel for nn_Assigner (3D IoU anchor assignment).

Strategy: split the 64 GT boxes across the 8 NeuronCores (8 GTs each); every
core scans all 262144 anchors against its 8 GTs and emits two per-anchor
f32 accumulators:
  pos[a] = max_j (iou(a, m_j) >= 0.5) * (64 - m_j)   (0 if no positive)
  neg[a] = min_j (0.3*union - inter)                  (> 0 iff all iou < 0.3)
The host max/min-combines the 8 cores' accumulators and decodes the final
labels/bboxes with a trivial 64-entry table lookup.

The device computation mirrors the reference's f32 operation order exactly
(same min/max/sub/mul sequence), so threshold decisions match bit-for-bit
except within ~1 ulp of the 0.5/0.3 boundaries.
"""
import numpy as np

import concourse.bass as bass
import concourse.mybir as mybir
from concourse.bass_utils import run_bass_kernel_spmd

N = 262144          # anchors
M = 64              # gt boxes
P = 128             # partitions
NCHUNK = 2          # anchor chunks per core
F = N // (P * NCHUNK)   # free-dim elements per chunk (1024)
GPC = M // 8        # gts per core
EPS = 1e-7

_cache = {}


def _build():
    nc = bass.Bass()
    f32 = mybir.dt.float32
    pk = nc.declare_dram_parameter("pk", [NCHUNK, P, 7, F], f32, isOutput=False)
    gts = nc.declare_dram_parameter("gts", [8, GPC], f32, isOutput=False)
    pos = nc.declare_dram_parameter("pos", [NCHUNK, P, F], f32, isOutput=True)
    neg = nc.declare_dram_parameter("neg", [NCHUNK, P, F], f32, isOutput=True)

    Alu = mybir.AluOpType

    with (
        nc.sbuf_tensor([P, 8 * GPC], f32) as grep,
        nc.sbuf_tensor([P, NCHUNK, 7, F], f32) as pkt,
        nc.sbuf_tensor([P, F], f32) as t_uz,
        nc.sbuf_tensor([P, F], f32) as t_uy,
        nc.sbuf_tensor([P, F], f32) as t_ux,
        nc.sbuf_tensor([P, F], f32) as t_wz,
        nc.sbuf_tensor([P, F], f32) as t_wy,
        nc.sbuf_tensor([P, F], f32) as t_wx,
        nc.sbuf_tensor([P, NCHUNK, 2, F], f32) as t_pacc,
        nc.sbuf_tensor([P, NCHUNK, 2, F], f32) as t_nacc,
        nc.semaphore("s_gt") as s_gt,
        nc.semaphore("s_in") as s_in,
        nc.semaphore("s_v") as s_v,
        nc.semaphore("s_out") as s_out,
        nc.Block() as block,
    ):
        @block.gpsimd
        def _(gpsimd):
            gpsimd.dma_start(
                grep[:],
                gts[:].rearrange("a b -> (a b)")[None, :].partition_broadcast(P),
            ).then_inc(s_gt, 16)

        @block.sync
        def _(sync):
            for c in range(NCHUNK):
                sync.dma_start(pkt[:, c], pk[c]).then_inc(s_in, 16)
            for c in range(NCHUNK):
                sync.wait_ge(s_v, 2 * (c + 1))
                sync.dma_start(pos[c], t_pacc[:, c, (GPC - 1) % 2]).then_inc(s_out, 16)
                sync.dma_start(neg[c], t_nacc[:, c, (GPC - 1) % 2]).then_inc(s_out, 16)
            sync.wait_ge(s_out, 32 * NCHUNK)

        @block.vector
        def _(v):
            v.wait_ge(s_gt, 16)

            def col(field, j):
                i = field * GPC + j
                return grep[:, i:i + 1]

            for c in range(NCHUNK):
                v.wait_ge(s_in, 16 * (c + 1))
                pl = lambda k: pkt[:, c, k]
                for j in range(GPC):
                    # w_d = min(r_d, R_d) - max(s_d, S_d), per dim
                    v.tensor_scalar(t_uz[:], pl(0), col(0, j), None, Alu.max)
                    v.scalar_tensor_tensor(t_wz[:], pl(3), col(3, j), t_uz[:], Alu.min, Alu.subtract)
                    v.tensor_scalar(t_uy[:], pl(1), col(1, j), None, Alu.max)
                    v.scalar_tensor_tensor(t_wy[:], pl(4), col(4, j), t_uy[:], Alu.min, Alu.subtract)
                    v.tensor_scalar(t_ux[:], pl(2), col(2, j), None, Alu.max)
                    v.scalar_tensor_tensor(t_wx[:], pl(5), col(5, j), t_ux[:], Alu.min, Alu.subtract)
                    # p_d = relu(w_d)  (reuse u-tiles)
                    v.tensor_scalar(t_uz[:], t_wz[:], 0.0, None, Alu.max)
                    v.tensor_scalar(t_uy[:], t_wy[:], 0.0, None, Alu.max)
                    v.tensor_scalar(t_ux[:], t_wx[:], 0.0, None, Alu.max)
                    # inter = (pz*py)*px   (m1 -> wz, inter -> wy)
                    v.tensor_tensor(t_wz[:], t_uz[:], t_uy[:], Alu.mult)
                    v.tensor_tensor(t_wy[:], t_wz[:], t_ux[:], Alu.mult)
                    # asum = a1 + A2_j -> wx ; u = asum - inter -> uz ; u2 = u + eps -> uy
                    v.tensor_scalar(t_wx[:], pl(6), col(6, j), None, Alu.add)
                    v.tensor_tensor(t_uz[:], t_wx[:], t_wy[:], Alu.subtract)
                    v.tensor_scalar(t_uy[:], t_uz[:], EPS, None, Alu.add)
                    # d05n = 0.5*u2 - inter -> ux ; d03n = 0.3*u2 - inter
                    v.tensor_scalar(t_ux[:], t_uy[:], 0.5, t_wy[:], Alu.mult, Alu.subtract) \
                        if False else \
                        v.scalar_tensor_tensor(t_ux[:], t_uy[:], 0.5, t_wy[:], Alu.mult, Alu.subtract)
                    nacc_dst = t_nacc[:, c, 0] if j == 0 else t_wz[:]
                    v.scalar_tensor_tensor(nacc_dst, t_uy[:], 0.3, t_wy[:], Alu.mult, Alu.subtract)
                    # score = (d05n <= 0) * W_j
                    pacc_dst = t_pacc[:, c, 0] if j == 0 else t_uy[:]
                    v.tensor_scalar(pacc_dst, t_ux[:], 0.0, col(7, j), Alu.is_le, Alu.mult)
                    if j > 0:
                        src, dst = (j - 1) % 2, j % 2
                        i1 = v.tensor_tensor(t_pacc[:, c, dst], t_pacc[:, c, src], t_uy[:], Alu.max)
                        i2 = v.tensor_tensor(t_nacc[:, c, dst], t_nacc[:, c, src], t_wz[:], Alu.min)
                        if j == GPC - 1:
                            i1.then_inc(s_v, 1)
                            i2.then_inc(s_v, 1)
    return nc


def _host_pack(bboxes):
    b = np.ascontiguousarray(bboxes, dtype=np.float32)
    sz, sy, sx = b[:, 0], b[:, 1], b[:, 2]
    rz, ry, rx = b[:, 3], b[:, 4], b[:, 5]
    a1 = ((rz - sz) * (ry - sy)) * (rx - sx)
    planes = np.stack([sz, sy, sx, rz, ry, rx, a1], axis=0)  # [7, N]
    # packed[c, p, k, f] = planes[k, p*(NCHUNK*F) + c*F + f]
    packed = planes.reshape(7, P, NCHUNK, F).transpose(2, 1, 0, 3)
    return np.ascontiguousarray(packed, dtype=np.float32)


def _host_gts(gt_bboxes):
    g = np.ascontiguousarray(gt_bboxes, dtype=np.float32)
    A2 = ((g[:, 3] - g[:, 0]) * (g[:, 4] - g[:, 1])) * (g[:, 5] - g[:, 2])
    W = (M - np.arange(M)).astype(np.float32)  # 64 - m
    full = np.stack([g[:, 0], g[:, 1], g[:, 2], g[:, 3], g[:, 4], g[:, 5], A2, W], axis=0)
    return [np.ascontiguousarray(full[:, i * GPC:(i + 1) * GPC], dtype=np.float32)
            for i in range(8)]


def kernel(bboxes, gt_bboxes, gt_labels):
    bboxes = np.asarray(bboxes, dtype=np.float32)
    gt_bboxes = np.asarray(gt_bboxes, dtype=np.float32)
    gt_labels = np.asarray(gt_labels)

    if "nc" not in _cache:
        _cache["nc"] = _build()
    nc = _cache["nc"]

    packed = _host_pack(bboxes)
    gts_per_core = _host_gts(gt_bboxes)
    in_maps = [{"pk": packed, "gts": gts_per_core[i]} for i in range(8)]
    res = run_bass_kernel_spmd(nc, in_maps, core_ids=list(range(8)))

    def unpack(a):  # [NCHUNK, P, F] -> [N] with anchor = p*(NCHUNK*F) + c*F + f
        return np.ascontiguousarray(np.transpose(a, (1, 0, 2))).reshape(N)

    posg = np.maximum.reduce([unpack(res.results[i]["pos"]) for i in range(8)])
    negg = np.minimum.reduce([unpack(res.results[i]["neg"]) for i in range(8)])

    pos_mask = posg > 0
    neg_mask = negg > 0
    idx = (M - posg).astype(np.int64)
    idx[~pos_mask] = 0

    zero = np.zeros((), dtype=gt_labels.dtype)
    minus1 = np.full((), -1, dtype=gt_labels.dtype)
    labels = np.where(pos_mask, gt_labels[idx], np.where(neg_mask, zero, minus1)).astype(gt_labels.dtype)
    bbox_out = np.where(pos_mask[:, None], gt_bboxes[idx], np.float32(-1.0)).astype(np.float32)
    return labels, bbox_out


# revision 3
# speedup vs baseline: 1.1395x; 1.1395x over previous
"""Trainium2 Bass kernel for nn_Assigner (3D IoU anchor assignment).

Strategy: split the 64 GT boxes across the 8 NeuronCores (8 GTs each); every
core scans all 262144 anchors against its 8 GTs and emits two per-anchor
f32 accumulators:
  pos[a] = max_j (iou(a, m_j) >= 0.5) * (64 - m_j)   (0 if no positive)
  neg[a] = min_j (0.3*union - inter)                  (> 0 iff all iou < 0.3)
The host max/min-combines the 8 cores' accumulators and decodes the final
labels/bboxes with a trivial 64-entry table lookup.

The device computation mirrors the reference's f32 operation order exactly
(same min/max/sub/mul sequence), so threshold decisions match bit-for-bit
except within ~1 ulp of the 0.5/0.3 boundaries.
"""
import numpy as np

import concourse.bass as bass
import concourse.mybir as mybir
from concourse.bass_utils import run_bass_kernel_spmd

N = 262144          # anchors
M = 64              # gt boxes
P = 128             # partitions
NCHUNK = 2          # anchor chunks per core
F = N // (P * NCHUNK)   # free-dim elements per chunk (1024)
GPC = M // 8        # gts per core
EPS = 1e-7

_cache = {}
_run_kwargs = {}
_last_res = None


def _build():
    nc = bass.Bass()
    f32 = mybir.dt.float32
    pk = nc.declare_dram_parameter("pk", [NCHUNK, P, 7, F], f32, isOutput=False)
    gts = nc.declare_dram_parameter("gts", [8, GPC], f32, isOutput=False)
    pos = nc.declare_dram_parameter("pos", [NCHUNK, P, F], f32, isOutput=True)
    neg = nc.declare_dram_parameter("neg", [NCHUNK, P, F], f32, isOutput=True)

    Alu = mybir.AluOpType

    with (
        nc.sbuf_tensor([P, 8 * GPC], f32) as grep,
        nc.sbuf_tensor([P, NCHUNK, 7, F], f32) as pkt,
        nc.sbuf_tensor([P, F], f32) as t_uz,
        nc.sbuf_tensor([P, F], f32) as t_uy,
        nc.sbuf_tensor([P, F], f32) as t_ux,
        nc.sbuf_tensor([P, F], f32) as t_wz,
        nc.sbuf_tensor([P, F], f32) as t_wy,
        nc.sbuf_tensor([P, F], f32) as t_wx,
        nc.sbuf_tensor([P, NCHUNK, 2, F], f32) as t_pacc,
        nc.sbuf_tensor([P, NCHUNK, 2, F], f32) as t_nacc,
        nc.semaphore("s_gt") as s_gt,
        nc.semaphore("s_in") as s_in,
        nc.semaphore("s_v") as s_v,
        nc.semaphore("s_out") as s_out,
        nc.Block() as block,
    ):
        @block.gpsimd
        def _(gpsimd):
            gpsimd.dma_start(
                grep[:],
                gts[:].rearrange("a b -> (a b)")[None, :].partition_broadcast(P),
            ).then_inc(s_gt, 16)

        @block.sync
        def _(sync):
            for c in range(NCHUNK):
                sync.dma_start(pkt[:, c], pk[c]).then_inc(s_in, 16)
            for c in range(NCHUNK):
                sync.wait_ge(s_v, 2 * (c + 1))
                sync.dma_start(pos[c], t_pacc[:, c, (GPC - 1) % 2]).then_inc(s_out, 16)
                sync.dma_start(neg[c], t_nacc[:, c, (GPC - 1) % 2]).then_inc(s_out, 16)
            sync.wait_ge(s_out, 32 * NCHUNK)

        @block.vector
        def _(v):
            v.wait_ge(s_gt, 16)

            def col(field, j):
                i = field * GPC + j
                return grep[:, i:i + 1]

            for c in range(NCHUNK):
                v.wait_ge(s_in, 16 * (c + 1))
                pl = lambda k: pkt[:, c, k]
                for j in range(GPC):
                    # w_d = min(r_d, R_d) - max(s_d, S_d), per dim
                    v.tensor_scalar(t_uz[:], pl(0), col(0, j), None, Alu.max)
                    v.scalar_tensor_tensor(t_wz[:], pl(3), col(3, j), t_uz[:], Alu.min, Alu.subtract)
                    v.tensor_scalar(t_uy[:], pl(1), col(1, j), None, Alu.max)
                    v.scalar_tensor_tensor(t_wy[:], pl(4), col(4, j), t_uy[:], Alu.min, Alu.subtract)
                    v.tensor_scalar(t_ux[:], pl(2), col(2, j), None, Alu.max)
                    v.scalar_tensor_tensor(t_wx[:], pl(5), col(5, j), t_ux[:], Alu.min, Alu.subtract)
                    # p_d = relu(w_d)  (reuse u-tiles)
                    v.tensor_scalar(t_uz[:], t_wz[:], 0.0, None, Alu.max)
                    v.tensor_scalar(t_uy[:], t_wy[:], 0.0, None, Alu.max)
                    v.tensor_scalar(t_ux[:], t_wx[:], 0.0, None, Alu.max)
                    # inter = (pz*py)*px   (m1 -> wz, inter -> wy)
                    v.tensor_tensor(t_wz[:], t_uz[:], t_uy[:], Alu.mult)
                    v.tensor_tensor(t_wy[:], t_wz[:], t_ux[:], Alu.mult)
                    # asum = a1 + A2_j -> wx ; u = asum - inter -> uz ; u2 = u + eps -> uy
                    v.tensor_scalar(t_wx[:], pl(6), col(6, j), None, Alu.add)
                    v.tensor_tensor(t_uz[:], t_wx[:], t_wy[:], Alu.subtract)
                    v.tensor_scalar(t_uy[:], t_uz[:], EPS, None, Alu.add)
                    # d05n = 0.5*u2 - inter -> ux ; d03n = 0.3*u2 - inter
                    v.scalar_tensor_tensor(t_ux[:], t_uy[:], 0.5, t_wy[:], Alu.mult, Alu.subtract)
                    nacc_dst = t_nacc[:, c, 0] if j == 0 else t_wz[:]
                    v.scalar_tensor_tensor(nacc_dst, t_uy[:], 0.3, t_wy[:], Alu.mult, Alu.subtract)
                    # score = (d05n <= 0) * W_j
                    pacc_dst = t_pacc[:, c, 0] if j == 0 else t_uy[:]
                    v.tensor_scalar(pacc_dst, t_ux[:], 0.0, col(7, j), Alu.is_le, Alu.mult)
                    if j > 0:
                        src, dst = (j - 1) % 2, j % 2
                        i1 = v.tensor_tensor(t_pacc[:, c, dst], t_pacc[:, c, src], t_uy[:], Alu.max)
                        i2 = v.tensor_tensor(t_nacc[:, c, dst], t_nacc[:, c, src], t_wz[:], Alu.min)
                        if j == GPC - 1:
                            i1.then_inc(s_v, 1)
                            i2.then_inc(s_v, 1)
    return nc


def _host_pack(bboxes):
    b = np.ascontiguousarray(bboxes, dtype=np.float32)
    sz, sy, sx = b[:, 0], b[:, 1], b[:, 2]
    rz, ry, rx = b[:, 3], b[:, 4], b[:, 5]
    a1 = ((rz - sz) * (ry - sy)) * (rx - sx)
    planes = np.stack([sz, sy, sx, rz, ry, rx, a1], axis=0)  # [7, N]
    # packed[c, p, k, f] = planes[k, p*(NCHUNK*F) + c*F + f]
    packed = planes.reshape(7, P, NCHUNK, F).transpose(2, 1, 0, 3)
    return np.ascontiguousarray(packed, dtype=np.float32)


def _host_gts(gt_bboxes):
    g = np.ascontiguousarray(gt_bboxes, dtype=np.float32)
    A2 = ((g[:, 3] - g[:, 0]) * (g[:, 4] - g[:, 1])) * (g[:, 5] - g[:, 2])
    W = (M - np.arange(M)).astype(np.float32)  # 64 - m
    full = np.stack([g[:, 0], g[:, 1], g[:, 2], g[:, 3], g[:, 4], g[:, 5], A2, W], axis=0)
    return [np.ascontiguousarray(full[:, i * GPC:(i + 1) * GPC], dtype=np.float32)
            for i in range(8)]


def kernel(bboxes, gt_bboxes, gt_labels):
    bboxes = np.asarray(bboxes, dtype=np.float32)
    gt_bboxes = np.asarray(gt_bboxes, dtype=np.float32)
    gt_labels = np.asarray(gt_labels)

    if "nc" not in _cache:
        _cache["nc"] = _build()
    nc = _cache["nc"]

    packed = _host_pack(bboxes)
    gts_per_core = _host_gts(gt_bboxes)
    in_maps = [{"pk": packed, "gts": gts_per_core[i]} for i in range(8)]
    global _last_res
    res = run_bass_kernel_spmd(nc, in_maps, core_ids=list(range(8)), **_run_kwargs)
    _last_res = res

    def unpack(a):  # [NCHUNK, P, F] -> [N] with anchor = p*(NCHUNK*F) + c*F + f
        return np.ascontiguousarray(np.transpose(a, (1, 0, 2))).reshape(N)

    posg = np.maximum.reduce([unpack(res.results[i]["pos"]) for i in range(8)])
    negg = np.minimum.reduce([unpack(res.results[i]["neg"]) for i in range(8)])

    pos_mask = posg > 0
    neg_mask = negg > 0
    idx = (M - posg).astype(np.int64)
    idx[~pos_mask] = 0

    zero = np.zeros((), dtype=gt_labels.dtype)
    minus1 = np.full((), -1, dtype=gt_labels.dtype)
    labels = np.where(pos_mask, gt_labels[idx], np.where(neg_mask, zero, minus1)).astype(gt_labels.dtype)
    bbox_out = np.where(pos_mask[:, None], gt_bboxes[idx], np.float32(-1.0)).astype(np.float32)
    return labels, bbox_out
